# revision 1
# baseline (speedup 1.0000x reference)
"""Causal self-attention (B=2, T=2048, C=2048, H=16) on 8 TRN2 NeuronCores.

Sharding: tensor-parallel over heads (2 heads per core, both batches on every
core). Each core computes q/k/v projections for its 2 heads, RoPE, causal
softmax(qk^T)v, and a partial output projection against its slice of Wo's
columns. The host sums the 8 partial projections and adds the (linear) bias
terms.

Layout strategy on device (all matmuls in float32r, ~1 cycle/row):
  - x is pre-transposed on host to xT[b] = x[b].T [C, T]; contraction over C
    has C on partitions for both operands.
  - q, k are produced transposed: qT/kT [head_dim, T] (head_dim on
    partitions). Scores are computed as S_T = kT_tile.T @ qT [keys, queries],
    so softmax normalization is along the partition (key) dim.
  - exp via ACT (scale = 1/sqrt(HD) fused); causal masking via 0/1 mask
    multiply on the 4 diagonal-boundary tiles per query block.
  - denominator: P_T tiles are summed (DVE) then one matmul with an
    all-ones [128,128] lhsT broadcasts column sums to all partitions.
  - y is produced transposed directly: yT = v_tile.T @ P_T [head_dim,
    queries]; normalization multiplies by the broadcast reciprocal.
  - output projection: out_tile = yT_slice.T @ woT_slice [T, C_out] — both
    operands already in the right layout; no transposes anywhere.
  - RoPE rotation (half-swap with sign) is a [128,128] permutation matmul;
    cos/sin tables are host-computed to mirror the reference bit-for-bit.
"""

import sys

sys.path.insert(0, "/opt/trn_rl_repo")

import numpy as np

import concourse.bacc as bacc
import concourse.mybir as mybir
import concourse.tile as tile
from concourse import bass_utils

B, T, C, H = 2, 2048, 2048, 16
HD = C // H  # 128
BASE = 10000.0
NC_ = 8  # cores
NH = H // NC_  # heads per core = 2
TB = 512  # T block
NTB = T // TB  # 4
CK = C // 128  # 16 contraction chunks
SCALE = 1.0 / float(np.sqrt(np.float32(HD)))

f32 = mybir.dt.float32
f32r = mybir.dt.float32r
AF = mybir.ActivationFunctionType
OP = mybir.AluOpType

TRACE = False
LAST_RESULT = None
LDW_OPT = False

_orig_run_command = bass_utils.run_command


def _patched_run_command(cmd, **kw):
    if LDW_OPT and isinstance(cmd, list):
        cmd = [
            ("--enable-ldw-opt=true" if c == "--enable-ldw-opt=false" else c)
            for c in cmd
        ]
    return _orig_run_command(cmd, **kw)


bass_utils.run_command = _patched_run_command

_STATE = {}


def _rope_tables():
    """cos/sin tables [HD, T] mirroring reference._rope_tables (f32 chain)."""
    try:
        import jax
        import jax.numpy as jnp

        cpu = jax.devices("cpu")[0]
        with jax.default_device(cpu):
            p = jnp.arange(HD // 2, dtype=jnp.float32)
            theta = jnp.power(BASE, -(2.0**p) / HD)
            pos = jnp.arange(1, T + 1, dtype=jnp.float32)[:, None]
            c = pos * theta
            ang = jnp.concatenate([c, c], axis=-1)  # [T, HD]
            cos = np.asarray(jnp.cos(ang)).T  # [HD, T]
            sin = np.asarray(jnp.sin(ang)).T
        return np.ascontiguousarray(cos), np.ascontiguousarray(sin)
    except Exception:
        p = np.arange(HD // 2, dtype=np.float32)
        theta = np.power(np.float32(BASE), (-(2.0**p) / HD).astype(np.float32))
        pos = np.arange(1, T + 1, dtype=np.float32)[:, None]
        c = (pos * theta).astype(np.float32)
        ang = np.concatenate([c, c], axis=-1)
        return (
            np.ascontiguousarray(np.cos(ang).T.astype(np.float32)),
            np.ascontiguousarray(np.sin(ang).T.astype(np.float32)),
        )


def _build_program():
    nc = bacc.Bacc("TRN2", target_bir_lowering=False, debug=False, num_devices=NC_)

    d_xT = nc.dram_tensor("xT", (B, C, T), f32, kind="ExternalInput")
    d_wq = nc.dram_tensor("wq", (C, NH * HD), f32, kind="ExternalInput")
    d_wk = nc.dram_tensor("wk", (C, NH * HD), f32, kind="ExternalInput")
    d_wv = nc.dram_tensor("wv", (C, NH * HD), f32, kind="ExternalInput")
    d_wo = nc.dram_tensor("wo", (NH * HD, C), f32, kind="ExternalInput")
    d_bq = nc.dram_tensor("bq", (HD, NH), f32, kind="ExternalInput")
    d_bk = nc.dram_tensor("bk", (HD, NH), f32, kind="ExternalInput")
    d_cos = nc.dram_tensor("cosT", (HD, T), f32, kind="ExternalInput")
    d_sin = nc.dram_tensor("sinT", (HD, T), f32, kind="ExternalInput")
    d_masks = nc.dram_tensor("masks", (4, 128, TB), f32, kind="ExternalInput")
    d_ones = nc.dram_tensor("onesm", (128, 128), f32, kind="ExternalInput")
    d_out = nc.dram_tensor("out", (B, T, C), f32, kind="ExternalOutput")

    with tile.TileContext(nc) as tc:
        with (
            tc.tile_pool(name="w", bufs=1) as wp,
            tc.tile_pool(name="xp", bufs=1) as xp,
            tc.tile_pool(name="kv", bufs=1) as kvp,
            tc.tile_pool(name="work", bufs=1) as wk_,
            tc.tile_pool(name="ps", bufs=1, space="PSUM") as ps,
        ):
            # --- resident weights/constants (f32r via gpsimd casting DMA) ---
            wq_t = wp.tile([128, CK, NH * HD], f32r, name="wq_t")
            wk_t = wp.tile([128, CK, NH * HD], f32r, name="wk_t")
            wv_t = wp.tile([128, CK, NH * HD], f32r, name="wv_t")
            for kc in range(CK):
                sl = slice(kc * 128, (kc + 1) * 128)
                nc.gpsimd.dma_start(wq_t[:, kc, :], d_wq.ap()[sl, :])
                nc.gpsimd.dma_start(wk_t[:, kc, :], d_wk.ap()[sl, :])
            wo_t = wp.tile([128, NH, C], f32r, name="wo_t")
            ones_t = wp.tile([128, 128], f32r, name="ones_t")
            nc.gpsimd.dma_start(ones_t[:], d_ones.ap()[:])
            bq_t = wp.tile([128, NH], f32, name="bq_t")
            nc.sync.dma_start(bq_t[:], d_bq.ap()[:])
            bk_t = wp.tile([128, NH], f32, name="bk_t")
            nc.sync.dma_start(bk_t[:], d_bk.ap()[:])
            masks_t = wp.tile([128, 128], f32, name="masks_t")
            nc.sync.dma_start(masks_t[:], d_masks.ap()[0, :, :128])

            for b in range(B):
                kts = [
                    kvp.tile([128, T], f32r, tag=f"kt{h}", name=f"kt{h}_{b}")
                    for h in range(NH)
                ]
                vt = kvp.tile([128, CK, NH * HD], f32r, tag="v", name=f"v_{b}")

                for tb in range(NTB):
                    tbs = slice(tb * TB, (tb + 1) * TB)
                    # ---- stream x block (16 casting DMAs of [128, 512]) ----
                    xts = []
                    for j in range(8):
                        xt = xp.tile([128, 2, TB], f32r, tag="xt", bufs=13)
                        for kcl in range(2):
                            kc = j * 2 + kcl
                            nc.gpsimd.dma_start(
                                xt[:, kcl, :],
                                d_xT.ap()[b, kc * 128 : (kc + 1) * 128, tbs],
                            )
                        xts.append(xt)
                    if b == 0 and tb == 0:
                        # deferred weight loads: wv needed at first v-proj,
                        # wo only at the first output projection
                        for kc in range(CK):
                            nc.gpsimd.dma_start(
                                wv_t[:, kc, :],
                                d_wv.ap()[kc * 128 : (kc + 1) * 128, :],
                            )
                        for h in range(NH):
                            nc.gpsimd.dma_start(
                                wo_t[:, h, :], d_wo.ap()[h * 128 : (h + 1) * 128, :]
                            )
                    cos_t = wk_.tile([128, TB], f32, tag="cs", bufs=2)
                    nc.sync.dma_start(cos_t[:], d_cos.ap()[:, tbs])
                    sin_t = wk_.tile([128, TB], f32, tag="cs", bufs=2)
                    nc.sync.dma_start(sin_t[:], d_sin.ap()[:, tbs])

                    def xc(kc):
                        return xts[kc // 2][:, kc % 2, :]

                    # ---- q/k projections + RoPE for both heads ----
                    qTs = []
                    for h in range(NH):
                        hsl = slice(h * HD, (h + 1) * HD)
                        qT = wk_.tile([128, TB], f32r, tag=f"q{h}", bufs=2)
                        for (w_t, b_t, dest) in (
                            (wq_t, bq_t, qT),
                            (wk_t, bk_t, kts[h][:, tbs]),
                        ):
                            prj = ps.tile([128, TB], f32, tag="mm", bufs=4)
                            for kc in range(CK):
                                nc.tensor.matmul(
                                    prj[:],
                                    w_t[:, kc, hsl],
                                    xc(kc),
                                    start=(kc == 0),
                                    stop=(kc == CK - 1),
                                )
                            qb = wk_.tile([128, TB], f32r, tag="qb", bufs=2)
                            nc.scalar.activation(
                                qb[:], prj[:], AF.Identity, bias=b_t[:, h : h + 1]
                            )
                            rotb = wk_.tile([128, TB], f32r, tag="rtmp", bufs=4)
                            nc.sync.dma_start(rotb[0:64, :], qb[64:128, :])
                            nc.sync.dma_start(rotb[64:128, :], qb[0:64, :])
                            t1 = wk_.tile([128, TB], f32, tag="rtmp", bufs=4)
                            nc.vector.tensor_tensor(
                                t1[:], qb[:].bitcast(f32), cos_t[:], OP.mult
                            )
                            t2 = wk_.tile([128, TB], f32, tag="rtmp", bufs=4)
                            nc.vector.tensor_tensor(
                                t2[:], rotb[:].bitcast(f32), sin_t[:], OP.mult
                            )
                            nc.vector.tensor_tensor(dest, t1[:], t2[:], OP.add)
                        qTs.append(qT)

                    # ---- v projection (both heads together, N=256) ----
                    for tt in range(4):
                        vps = ps.tile([128, NH * HD], f32, tag="mm", bufs=4)
                        for kc in range(CK):
                            nc.tensor.matmul(
                                vps[:],
                                xc(kc)[:, tt * 128 : (tt + 1) * 128],
                                wv_t[:, kc, :],
                                start=(kc == 0),
                                stop=(kc == CK - 1),
                            )
                        if tt % 2 == 0:
                            nc.scalar.activation(
                                vt[:, tb * 4 + tt, :], vps[:], AF.Identity
                            )
                        else:
                            nc.vector.tensor_copy(vt[:, tb * 4 + tt, :], vps[:])

                    # ---- attention for this query block ----
                    yTs = []
                    nkt = 4 * tb + 4
                    for h in range(NH):
                        hsl = slice(h * HD, (h + 1) * HD)
                        y_ps = ps.tile([128, TB], f32, tag="my", bufs=2)
                        dacc = wk_.tile([128, TB], f32r, tag="dacc", bufs=2)
                        for kt in range(nkt):
                            # causal trim: key tile kt only reaches queries
                            # >= q0 = 128*(kt-4*tb); skip the dead columns
                            o = kt - 4 * tb
                            q0 = 128 * o if o > 0 else 0
                            W = TB - q0
                            s_ps = ps.tile([128, TB], f32, tag="ms", bufs=2)
                            nc.tensor.matmul(
                                s_ps[:, :W],
                                kts[h][:, kt * 128 : (kt + 1) * 128],
                                qTs[h][:, q0:],
                            )
                            pt = wk_.tile([128, TB], f32r, tag="p", bufs=4)
                            nc.scalar.activation(
                                pt[:, :W], s_ps[:, :W], AF.Exp, scale=SCALE
                            )
                            if o >= 0:
                                # triangular chunk is always the first 128
                                # live columns; its mask equals mask0[:, :128]
                                nc.vector.tensor_tensor(
                                    pt[:, :128],
                                    pt[:, :128].bitcast(f32),
                                    masks_t[:],
                                    OP.mult,
                                )
                            nc.tensor.matmul(
                                y_ps[:, q0:],
                                vt[:, kt, hsl],
                                pt[:, :W],
                                start=(kt == 0),
                                stop=(kt == nkt - 1),
                                skip_group_check=True,
                            )
                            if kt == 0:
                                nc.vector.tensor_copy(dacc[:], pt[:].bitcast(f32))
                            else:
                                nc.vector.tensor_tensor(
                                    dacc[:, q0:],
                                    dacc[:, q0:].bitcast(f32),
                                    pt[:, :W].bitcast(f32),
                                    OP.add,
                                )
                        den_ps = ps.tile([128, TB], f32, tag="mm", bufs=4)
                        nc.tensor.matmul(den_ps[:], ones_t[:], dacc[:])
                        rden = wk_.tile([128, TB], f32, tag="rden", bufs=2)
                        nc.vector.reciprocal_approx_fast(rden[:], den_ps[:])
                        yT = wk_.tile([128, TB], f32r, tag="y", bufs=3)
                        nc.vector.tensor_tensor(yT[:], y_ps[:], rden[:], OP.mult)
                        yTs.append(yT)
                    # ---- partial output projection ----
                    for tt in range(4):
                        r0 = tb * TB + tt * 128
                        for ncc in range(4):
                            o_ps = ps.tile([128, TB], f32, tag="mm", bufs=4)
                            for h in range(NH):
                                nc.tensor.matmul(
                                    o_ps[:],
                                    yTs[h][:, tt * 128 : (tt + 1) * 128],
                                    wo_t[:, h, ncc * TB : (ncc + 1) * TB],
                                    start=(h == 0),
                                    stop=(h == NH - 1),
                                )
                            ot = wk_.tile([128, TB], f32, tag="o", bufs=3)
                            if ncc % 2 == 0:
                                nc.scalar.activation(ot[:], o_ps[:], AF.Identity)
                            else:
                                nc.vector.tensor_copy(ot[:], o_ps[:])
                            nc.sync.dma_start(
                                d_out.ap()[
                                    b, r0 : r0 + 128, ncc * TB : (ncc + 1) * TB
                                ],
                                ot[:],
                            )

    nc.compile()
    return nc


def _get_program():
    if "nc" not in _STATE:
        _STATE["nc"] = _build_program()
    return _STATE["nc"]


def _enable_trace_hooks():
    import types

    import antenv

    if not hasattr(antenv, "axon_hooks"):
        hooks_mod = types.ModuleType("antenv.axon_hooks")
        _hook = [None]
        hooks_mod.set_axon_ntff_profile_hook = lambda h: _hook.__setitem__(0, h)
        hooks_mod.get_axon_ntff_profile_hook = lambda: _hook[0]
        sys.modules["antenv.axon_hooks"] = hooks_mod
        antenv.axon_hooks = hooks_mod
        from trn_agent_boot.trn_boot import _ntff_profile_via_ctypes

        hooks_mod.set_axon_ntff_profile_hook(
            _ntff_profile_via_ctypes("/opt/axon/libaxon_pjrt.so")
        )
    bass_utils.upload_artifacts = lambda tmpdir: f"local://{tmpdir}"


def kernel(x, Wqkv, bqkv, Wo, bo):
    global LAST_RESULT
    x = np.asarray(x, dtype=np.float32)
    Wqkv = np.asarray(Wqkv, dtype=np.float32)
    bqkv = np.asarray(bqkv, dtype=np.float32)
    Wo = np.asarray(Wo, dtype=np.float32)
    bo = np.asarray(bo, dtype=np.float32)

    nc = _get_program()

    cosT, sinT = _rope_tables()
    sinT = sinT.copy()
    sinT[: HD // 2, :] *= -1.0  # rotation sign folded into the sin table
    onesm = np.ones((128, 128), dtype=np.float32)
    # masks[o][j, i] = 1 if key (kt*128+j) <= query (tb*512+i), o = kt-4*tb
    i_idx = np.arange(TB)[None, :]
    j_idx = np.arange(128)[:, None]
    masks = np.stack(
        [(j_idx <= i_idx - 128 * o).astype(np.float32) for o in range(4)]
    )
    xT = np.ascontiguousarray(x.transpose(0, 2, 1))

    in_maps = []
    for c in range(NC_):
        rs = slice(c * NH * HD, (c + 1) * NH * HD)
        in_maps.append(
            {
                "xT": xT,
                "wq": np.ascontiguousarray(Wqkv[0 * C :][rs.start : rs.stop, :].T),
                "wk": np.ascontiguousarray(Wqkv[1 * C :][rs.start : rs.stop, :].T),
                "wv": np.ascontiguousarray(Wqkv[2 * C :][rs.start : rs.stop, :].T),
                "wo": np.ascontiguousarray(Wo[:, rs].T),
                "bq": np.ascontiguousarray(bqkv[0 * C :][rs].reshape(NH, HD).T),
                "bk": np.ascontiguousarray(bqkv[1 * C :][rs].reshape(NH, HD).T),
                "cosT": cosT,
                "sinT": sinT,
                "masks": masks,
                "onesm": onesm,
            }
        )

    if TRACE:
        _enable_trace_hooks()
    res = bass_utils.run_bass_kernel_spmd(
        nc, in_maps, core_ids=list(range(NC_)), trace=TRACE
    )
    LAST_RESULT = res

    out = np.zeros((B, T, C), dtype=np.float64)
    for c in range(NC_):
        out += res.results[c]["out"]
    bv = bqkv[2 * C : 3 * C]
    out += (bo + Wo @ bv)[None, None, :]
    return out.astype(np.float32)



# revision 2
# speedup vs baseline: 1.3417x; 1.3417x over previous
"""Causal self-attention (B=2, T=2048, C=2048, H=16) on 8 TRN2 NeuronCores.

Sharding: tensor-parallel over heads (2 heads per core, both batches on every
core). Each core computes q/k/v projections for its 2 heads, RoPE, causal
softmax(qk^T)v, and a partial output projection against its slice of Wo's
columns. The host sums the 8 partial projections and adds the (linear) bias
terms.

v2: all matmul operands are bf16 (PSUM accumulation stays fp32). fp32
LDWEIGHTS gets no Fast Weight Load and dominated the v1 PE timeline; bf16
halves both the weight-load path and all DMA traffic. Inputs are pre-cast
and pre-tiled on host so every weight/x transfer is one large DMA.

Layout on device:
  - x is host-relaid to xR[b] = [128, CK, T] (partition = c%128within chunk);
    contraction over C has C on partitions for both operands.
  - q, k are produced transposed: qT/kT [head_dim, T]. Scores are
    S_T = kT_tile.T @ qT [keys, queries]; softmax normalization runs along
    the partition (key) dim.
  - exp via ACT (scale = 1/sqrt(HD) fused), bf16 out; causal masking via a
    0/1 bf16 mask multiply on the diagonal-boundary tiles.
  - denominator: P_T tiles are accumulated on DVE (bf16, 2x mode), then one
    matmul with an all-ones [128,128] lhsT broadcasts fp32 column sums.
  - y is produced transposed: yT = v_tile.T @ P_T; normalization multiplies
    by the broadcast reciprocal.
  - output projection: out_tile = yT_slice.T @ woT_slice; a [128, 2048]
    bf16 staging tile per row block keeps the writeback to 1 DMA.
  - RoPE rotation (half-swap, sign folded into the sin table) is 2 SBUF
    partition-swap DMAs + 3 bf16 DVE multiplies/adds.
"""

import sys

sys.path.insert(0, "/opt/trn_rl_repo")

import ml_dtypes
import numpy as np

import concourse.bacc as bacc
import concourse.mybir as mybir
import concourse.tile as tile
from concourse import bass_utils

B, T, C, H = 2, 2048, 2048, 16
HD = C // H  # 128
BASE = 10000.0
NC_ = 8  # cores
NH = H // NC_  # heads per core = 2
TB = 512  # T block
NTB = T // TB  # 4
CK = C // 128  # 16 contraction chunks
SCALE = 1.0 / float(np.sqrt(np.float32(HD)))

f32 = mybir.dt.float32
bf16 = mybir.dt.bfloat16
AF = mybir.ActivationFunctionType
OP = mybir.AluOpType
BF = ml_dtypes.bfloat16

TRACE = False
LAST_RESULT = None
LDW_OPT = False

_orig_run_command = bass_utils.run_command


def _patched_run_command(cmd, **kw):
    if LDW_OPT and isinstance(cmd, list):
        cmd = [
            ("--enable-ldw-opt=true" if c == "--enable-ldw-opt=false" else c)
            for c in cmd
        ]
    return _orig_run_command(cmd, **kw)


bass_utils.run_command = _patched_run_command

_STATE = {}


def _rope_tables():
    """cos/sin tables [HD, T] mirroring reference._rope_tables (f32 chain)."""
    try:
        import jax
        import jax.numpy as jnp

        cpu = jax.devices("cpu")[0]
        with jax.default_device(cpu):
            p = jnp.arange(HD // 2, dtype=jnp.float32)
            theta = jnp.power(BASE, -(2.0**p) / HD)
            pos = jnp.arange(1, T + 1, dtype=jnp.float32)[:, None]
            c = pos * theta
            ang = jnp.concatenate([c, c], axis=-1)  # [T, HD]
            cos = np.asarray(jnp.cos(ang)).T  # [HD, T]
            sin = np.asarray(jnp.sin(ang)).T
        return np.ascontiguousarray(cos), np.ascontiguousarray(sin)
    except Exception:
        p = np.arange(HD // 2, dtype=np.float32)
        theta = np.power(np.float32(BASE), (-(2.0**p) / HD).astype(np.float32))
        pos = np.arange(1, T + 1, dtype=np.float32)[:, None]
        c = (pos * theta).astype(np.float32)
        ang = np.concatenate([c, c], axis=-1)
        return (
            np.ascontiguousarray(np.cos(ang).T.astype(np.float32)),
            np.ascontiguousarray(np.sin(ang).T.astype(np.float32)),
        )


def _build_program():
    nc = bacc.Bacc("TRN2", target_bir_lowering=False, debug=False, num_devices=NC_)

    d_x = nc.dram_tensor("xR", (B, 128, CK, T), bf16, kind="ExternalInput")
    d_wq = nc.dram_tensor("wq", (128, CK, NH * HD), bf16, kind="ExternalInput")
    d_wk = nc.dram_tensor("wk", (128, CK, NH * HD), bf16, kind="ExternalInput")
    d_wv = nc.dram_tensor("wv", (128, CK, NH * HD), bf16, kind="ExternalInput")
    d_wo = nc.dram_tensor("wo", (128, NH, C), bf16, kind="ExternalInput")
    d_bq = nc.dram_tensor("bq", (HD, NH), f32, kind="ExternalInput")
    d_bk = nc.dram_tensor("bk", (HD, NH), f32, kind="ExternalInput")
    d_cos = nc.dram_tensor("cosT", (HD, T), bf16, kind="ExternalInput")
    d_sin = nc.dram_tensor("sinT", (HD, T), bf16, kind="ExternalInput")
    d_mask = nc.dram_tensor("mask0", (128, 128), bf16, kind="ExternalInput")
    d_ones = nc.dram_tensor("onesm", (128, 128), bf16, kind="ExternalInput")
    d_out = nc.dram_tensor("out", (B, T, C), bf16, kind="ExternalOutput")

    with tile.TileContext(nc) as tc:
        with (
            tc.tile_pool(name="w", bufs=1) as wp,
            tc.tile_pool(name="xp", bufs=1) as xp,
            tc.tile_pool(name="kv", bufs=1) as kvp,
            tc.tile_pool(name="work", bufs=1) as wk_,
            tc.tile_pool(name="ps", bufs=1, space="PSUM") as ps,
        ):
            # --- resident weights/constants ---
            wq_t = wp.tile([128, CK, NH * HD], bf16, name="wq_t")
            wk_t = wp.tile([128, CK, NH * HD], bf16, name="wk_t")
            nc.sync.dma_start(wq_t[:], d_wq.ap()[:])
            nc.sync.dma_start(wk_t[:], d_wk.ap()[:])
            bq_t = wp.tile([128, NH], f32, name="bq_t")
            nc.sync.dma_start(bq_t[:], d_bq.ap()[:])
            bk_t = wp.tile([128, NH], f32, name="bk_t")
            nc.sync.dma_start(bk_t[:], d_bk.ap()[:])

            wv_t = wp.tile([128, CK, NH * HD], bf16, name="wv_t")
            wo_t = wp.tile([128, NH, C], bf16, name="wo_t")
            cos_t = wp.tile([128, T], bf16, name="cos_t")
            sin_t = wp.tile([128, T], bf16, name="sin_t")
            masks_t = wp.tile([128, 128], bf16, name="masks_t")
            ones_t = wp.tile([128, 128], bf16, name="ones_t")
            nc.gpsimd.dma_start(cos_t[:], d_cos.ap()[:])
            nc.gpsimd.dma_start(sin_t[:], d_sin.ap()[:])
            nc.gpsimd.dma_start(masks_t[:], d_mask.ap()[:])
            nc.gpsimd.dma_start(ones_t[:], d_ones.ap()[:])

            for b in range(B):
                kts = [
                    kvp.tile([128, T], bf16, tag=f"kt{h}", bufs=2, name=f"kt{h}_{b}")
                    for h in range(NH)
                ]
                vt = kvp.tile([128, CK, NH * HD], bf16, tag="v", bufs=2, name=f"v_{b}")

                for tb in range(NTB):
                    tbs = slice(tb * TB, (tb + 1) * TB)
                    # ---- stream x block (4 DMAs of [128, 4, 512]) ----
                    xts = []
                    for j in range(4):
                        xt = xp.tile([128, 4, TB], bf16, tag="xt", bufs=12)
                        eng = nc.sync if (b == 0 and tb == 0) else nc.gpsimd
                        eng.dma_start(
                            xt[:], d_x.ap()[b, :, 4 * j : 4 * j + 4, tbs]
                        )
                        xts.append(xt)
                    if b == 0 and tb == 0:
                        # deferred weight loads: wv needed at first v-proj,
                        # wo only at the first output projection
                        nc.gpsimd.dma_start(wv_t[:], d_wv.ap()[:])
                        nc.gpsimd.dma_start(wo_t[:], d_wo.ap()[:])

                    def xc(kc):
                        return xts[kc // 4][:, kc % 4, :]

                    # ---- q/k projections + RoPE for both heads ----
                    qTs = []
                    for h in range(NH):
                        hsl = slice(h * HD, (h + 1) * HD)
                        qT = wk_.tile([128, TB], bf16, tag=f"q{h}", bufs=2)
                        for (w_t, b_t, dest) in (
                            (wq_t, bq_t, qT),
                            (wk_t, bk_t, kts[h][:, tbs]),
                        ):
                            prj = ps.tile([128, TB], f32, tag="mm", bufs=2)
                            for kc in range(CK):
                                nc.tensor.matmul(
                                    prj[:],
                                    w_t[:, kc, hsl],
                                    xc(kc),
                                    start=(kc == 0),
                                    stop=(kc == CK - 1),
                                )
                            qb = wk_.tile([128, TB], bf16, tag="qb", bufs=3)
                            nc.scalar.activation(
                                qb[:], prj[:], AF.Identity, bias=b_t[:, h : h + 1]
                            )
                            rotb = wk_.tile([128, TB], bf16, tag="rot", bufs=3)
                            nc.sync.dma_start(rotb[0:64, :], qb[64:128, :])
                            nc.sync.dma_start(rotb[64:128, :], qb[0:64, :])
                            t1 = wk_.tile([128, TB], bf16, tag="t1", bufs=3)
                            nc.vector.tensor_tensor(
                                t1[:], qb[:], cos_t[:, tbs], OP.mult
                            )
                            t2 = wk_.tile([128, TB], bf16, tag="t2", bufs=3)
                            nc.vector.tensor_tensor(
                                t2[:], rotb[:], sin_t[:, tbs], OP.mult
                            )
                            nc.vector.tensor_tensor(dest, t1[:], t2[:], OP.add)
                        qTs.append(qT)

                    # ---- v projection (both heads together, N=256) ----
                    for tt in range(4):
                        vps = ps.tile([128, NH * HD], f32, tag="mm", bufs=2)
                        for kc in range(CK):
                            nc.tensor.matmul(
                                vps[:],
                                xc(kc)[:, tt * 128 : (tt + 1) * 128],
                                wv_t[:, kc, :],
                                start=(kc == 0),
                                stop=(kc == CK - 1),
                            )
                        if tt % 2 == 0:
                            nc.scalar.activation(
                                vt[:, tb * 4 + tt, :], vps[:], AF.Identity
                            )
                        else:
                            nc.vector.tensor_copy(vt[:, tb * 4 + tt, :], vps[:])

                    # ---- attention for this query block ----
                    yTs = []
                    nkt = 4 * tb + 4
                    for h in range(NH):
                        hsl = slice(h * HD, (h + 1) * HD)
                        y_ps = ps.tile([128, TB], f32, tag="my", bufs=2)
                        dacc = wk_.tile([128, TB], bf16, tag="dacc", bufs=2)
                        for kt in range(nkt):
                            # causal trim: key tile kt only reaches queries
                            # >= q0 = 128*(kt-4*tb); skip the dead columns
                            o = kt - 4 * tb
                            q0 = 128 * o if o > 0 else 0
                            W = TB - q0
                            s_ps = ps.tile([128, TB], f32, tag="ms", bufs=2)
                            nc.tensor.matmul(
                                s_ps[:, :W],
                                kts[h][:, kt * 128 : (kt + 1) * 128],
                                qTs[h][:, q0:],
                            )
                            pt = wk_.tile([128, TB], bf16, tag="p", bufs=6)
                            nc.scalar.activation(
                                pt[:, :W], s_ps[:, :W], AF.Exp, scale=SCALE
                            )
                            if o >= 0:
                                # triangular chunk is always the first 128
                                # live columns; its mask equals mask0
                                nc.vector.tensor_tensor(
                                    pt[:, :128],
                                    pt[:, :128],
                                    masks_t[:],
                                    OP.mult,
                                )
                            nc.tensor.matmul(
                                y_ps[:, q0:],
                                vt[:, kt, hsl],
                                pt[:, :W],
                                start=(kt == 0),
                                stop=(kt == nkt - 1),
                                skip_group_check=True,
                            )
                            if kt == 0:
                                nc.vector.tensor_copy(dacc[:], pt[:])
                            else:
                                nc.vector.tensor_tensor(
                                    dacc[:, q0:],
                                    dacc[:, q0:],
                                    pt[:, :W],
                                    OP.add,
                                )
                        den_ps = ps.tile([128, TB], f32, tag="mm", bufs=2)
                        nc.tensor.matmul(den_ps[:], ones_t[:], dacc[:])
                        rden = wk_.tile([128, TB], f32, tag="rden", bufs=2)
                        nc.vector.reciprocal_approx_fast(rden[:], den_ps[:])
                        yT = wk_.tile([128, TB], bf16, tag="y", bufs=3)
                        nc.vector.tensor_tensor(yT[:], y_ps[:], rden[:], OP.mult)
                        yTs.append(yT)
                    # ---- partial output projection ----
                    for tt in range(4):
                        r0 = tb * TB + tt * 128
                        ot = wk_.tile([128, 4, TB], bf16, tag="o", bufs=2)
                        for ncc in range(4):
                            o_ps = ps.tile([128, TB], f32, tag="mo", bufs=2)
                            for h in range(NH):
                                nc.tensor.matmul(
                                    o_ps[:],
                                    yTs[h][:, tt * 128 : (tt + 1) * 128],
                                    wo_t[:, h, ncc * TB : (ncc + 1) * TB],
                                    start=(h == 0),
                                    stop=(h == NH - 1),
                                )
                            if ncc % 2 == 0:
                                nc.scalar.activation(
                                    ot[:, ncc, :], o_ps[:], AF.Identity
                                )
                            else:
                                nc.vector.tensor_copy(ot[:, ncc, :], o_ps[:])
                        nc.sync.dma_start(
                            d_out.ap()[b, r0 : r0 + 128, :],
                            ot[:],
                        )

    nc.compile()
    return nc


def _get_program():
    if "nc" not in _STATE:
        _STATE["nc"] = _build_program()
    return _STATE["nc"]


def _enable_trace_hooks():
    import types

    import antenv

    if not hasattr(antenv, "axon_hooks"):
        hooks_mod = types.ModuleType("antenv.axon_hooks")
        _hook = [None]
        hooks_mod.set_axon_ntff_profile_hook = lambda h: _hook.__setitem__(0, h)
        hooks_mod.get_axon_ntff_profile_hook = lambda: _hook[0]
        sys.modules["antenv.axon_hooks"] = hooks_mod
        antenv.axon_hooks = hooks_mod
        from trn_agent_boot.trn_boot import _ntff_profile_via_ctypes

        hooks_mod.set_axon_ntff_profile_hook(
            _ntff_profile_via_ctypes("/opt/axon/libaxon_pjrt.so")
        )
    bass_utils.upload_artifacts = lambda tmpdir: f"local://{tmpdir}"


def kernel(x, Wqkv, bqkv, Wo, bo):
    global LAST_RESULT
    x = np.asarray(x, dtype=np.float32)
    Wqkv = np.asarray(Wqkv, dtype=np.float32)
    bqkv = np.asarray(bqkv, dtype=np.float32)
    Wo = np.asarray(Wo, dtype=np.float32)
    bo = np.asarray(bo, dtype=np.float32)

    nc = _get_program()

    cosT, sinT = _rope_tables()
    sinT = sinT.copy()
    sinT[: HD // 2, :] *= -1.0  # rotation sign folded into the sin table
    cosT = cosT.astype(BF)
    sinT = sinT.astype(BF)
    onesm = np.ones((128, 128), dtype=BF)
    # mask0[j, i] = 1 if key j <= query i within the diagonal 128-tile
    i_idx = np.arange(128)[None, :]
    j_idx = np.arange(128)[:, None]
    mask0 = (j_idx <= i_idx).astype(BF)
    # x -> [B, 128, CK, T]: partition p = c % 128 within chunk kc = c // 128
    xT = x.transpose(0, 2, 1)  # [B, C, T]
    xR = np.ascontiguousarray(
        xT.reshape(B, CK, 128, T).transpose(0, 2, 1, 3)
    ).astype(BF)

    def wtile(w):  # [rows=256, C] -> [128, CK, 256] (lhsT chunks)
        return np.ascontiguousarray(
            w.T.reshape(CK, 128, NH * HD).transpose(1, 0, 2)
        ).astype(BF)

    in_maps = []
    for c in range(NC_):
        rs = slice(c * NH * HD, (c + 1) * NH * HD)
        wo_c = Wo[:, rs].T  # [256, C]
        in_maps.append(
            {
                "xR": xR,
                "wq": wtile(Wqkv[0 * C :][rs.start : rs.stop, :]),
                "wk": wtile(Wqkv[1 * C :][rs.start : rs.stop, :]),
                "wv": wtile(Wqkv[2 * C :][rs.start : rs.stop, :]),
                "wo": np.ascontiguousarray(
                    wo_c.reshape(NH, 128, C).transpose(1, 0, 2)
                ).astype(BF),
                "bq": np.ascontiguousarray(bqkv[0 * C :][rs].reshape(NH, HD).T),
                "bk": np.ascontiguousarray(bqkv[1 * C :][rs].reshape(NH, HD).T),
                "cosT": cosT,
                "sinT": sinT,
                "mask0": mask0,
                "onesm": onesm,
            }
        )

    if TRACE:
        _enable_trace_hooks()
    res = bass_utils.run_bass_kernel_spmd(
        nc, in_maps, core_ids=list(range(NC_)), trace=TRACE
    )
    LAST_RESULT = res

    out = np.zeros((B, T, C), dtype=np.float64)
    for c in range(NC_):
        out += res.results[c]["out"].astype(np.float32)
    bv = bqkv[2 * C : 3 * C]
    out += (bo + Wo @ bv)[None, None, :]
    return out.astype(np.float32)


# revision 9
# speedup vs baseline: 1.5705x; 1.1705x over previous
"""Causal self-attention (B=2, T=2048, C=2048, H=16) on 8 TRN2 NeuronCores.

Sharding: tensor-parallel over heads (2 heads per core, both batches on every
core). Each core computes q/k/v projections for its 2 heads, RoPE, causal
softmax(qk^T)v, and a partial output projection against its slice of Wo's
columns. The host sums the 8 partial projections and adds the (linear) bias
terms.

v2: all matmul operands are bf16 (PSUM accumulation stays fp32). fp32
LDWEIGHTS gets no Fast Weight Load and dominated the v1 PE timeline; bf16
halves both the weight-load path and all DMA traffic. Inputs are pre-cast
and pre-tiled on host so every weight/x transfer is one large DMA.

Layout on device:
  - x is host-relaid to xR[b] = [128, CK, T] (partition = c%128within chunk);
    contraction over C has C on partitions for both operands.
  - q, k are produced transposed: qT/kT [head_dim, T]. Scores are
    S_T = kT_tile.T @ qT [keys, queries]; softmax normalization runs along
    the partition (key) dim.
  - exp via ACT (scale = 1/sqrt(HD) fused), bf16 out; causal masking via a
    0/1 bf16 mask multiply on the diagonal-boundary tiles.
  - denominator: P_T tiles are accumulated on DVE (bf16, 2x mode), then one
    matmul with an all-ones [128,128] lhsT broadcasts fp32 column sums.
  - y is produced transposed: yT = v_tile.T @ P_T; normalization multiplies
    by the broadcast reciprocal.
  - output projection: out_tile = yT_slice.T @ woT_slice; a [128, 2048]
    bf16 staging tile per row block keeps the writeback to 1 DMA.
  - RoPE rotation (half-swap, sign folded into the sin table) is 2 SBUF
    partition-swap DMAs + 3 bf16 DVE multiplies/adds.
"""

import sys

sys.path.insert(0, "/opt/trn_rl_repo")

import ml_dtypes
import numpy as np

import concourse.bacc as bacc
import concourse.mybir as mybir
import concourse.tile as tile
from concourse import bass_utils

B, T, C, H = 2, 2048, 2048, 16
HD = C // H  # 128
BASE = 10000.0
NC_ = 8  # cores
NH = H // NC_  # heads per core = 2
TB = 512  # T block
NTB = T // TB  # 4
CK = C // 128  # 16 contraction chunks
SCALE = 1.0 / float(np.sqrt(np.float32(HD)))

f32 = mybir.dt.float32
bf16 = mybir.dt.bfloat16
AF = mybir.ActivationFunctionType
OP = mybir.AluOpType
BF = ml_dtypes.bfloat16

TRACE = False
LAST_RESULT = None
LDW_OPT = False

_orig_run_command = bass_utils.run_command


def _patched_run_command(cmd, **kw):
    if LDW_OPT and isinstance(cmd, list):
        cmd = [
            ("--enable-ldw-opt=true" if c == "--enable-ldw-opt=false" else c)
            for c in cmd
        ]
    return _orig_run_command(cmd, **kw)


bass_utils.run_command = _patched_run_command

_STATE = {}


def _rope_tables():
    """cos/sin tables [HD, T] mirroring reference._rope_tables (f32 chain)."""
    try:
        import jax
        import jax.numpy as jnp

        cpu = jax.devices("cpu")[0]
        with jax.default_device(cpu):
            p = jnp.arange(HD // 2, dtype=jnp.float32)
            theta = jnp.power(BASE, -(2.0**p) / HD)
            pos = jnp.arange(1, T + 1, dtype=jnp.float32)[:, None]
            c = pos * theta
            ang = jnp.concatenate([c, c], axis=-1)  # [T, HD]
            cos = np.asarray(jnp.cos(ang)).T  # [HD, T]
            sin = np.asarray(jnp.sin(ang)).T
        return np.ascontiguousarray(cos), np.ascontiguousarray(sin)
    except Exception:
        p = np.arange(HD // 2, dtype=np.float32)
        theta = np.power(np.float32(BASE), (-(2.0**p) / HD).astype(np.float32))
        pos = np.arange(1, T + 1, dtype=np.float32)[:, None]
        c = (pos * theta).astype(np.float32)
        ang = np.concatenate([c, c], axis=-1)
        return (
            np.ascontiguousarray(np.cos(ang).T.astype(np.float32)),
            np.ascontiguousarray(np.sin(ang).T.astype(np.float32)),
        )


def _build_program():
    nc = bacc.Bacc("TRN2", target_bir_lowering=False, debug=False, num_devices=NC_)

    d_x = nc.dram_tensor("xR", (B, 128, CK, T), bf16, kind="ExternalInput")
    d_wq = nc.dram_tensor("wq", (128, CK, NH * HD), bf16, kind="ExternalInput")
    d_wk = nc.dram_tensor("wk", (128, CK, NH * HD), bf16, kind="ExternalInput")
    d_wv = nc.dram_tensor("wv", (128, CK, NH * HD), bf16, kind="ExternalInput")
    d_wo = nc.dram_tensor("wo", (128, NH, C), bf16, kind="ExternalInput")
    d_bq = nc.dram_tensor("bq", (HD, NH), f32, kind="ExternalInput")
    d_bk = nc.dram_tensor("bk", (HD, NH), f32, kind="ExternalInput")
    d_cos = nc.dram_tensor("cosT", (HD, T), bf16, kind="ExternalInput")
    d_sin = nc.dram_tensor("sinT", (HD, T), bf16, kind="ExternalInput")
    d_mask = nc.dram_tensor("mask0", (128, 128), bf16, kind="ExternalInput")
    d_ones = nc.dram_tensor("onesm", (128, 128), bf16, kind="ExternalInput")
    d_out = nc.dram_tensor("out", (B, T, C), bf16, kind="ExternalOutput")

    with tile.TileContext(nc) as tc:
        with (
            tc.tile_pool(name="w", bufs=1) as wp,
            tc.tile_pool(name="xp", bufs=1) as xp,
            tc.tile_pool(name="kv", bufs=1) as kvp,
            tc.tile_pool(name="work", bufs=1) as wk_,
            tc.tile_pool(name="ps", bufs=1, space="PSUM") as ps,
        ):
            # --- resident weights/constants ---
            wq_t = wp.tile([128, CK, NH * HD], bf16, name="wq_t")
            wk_t = wp.tile([128, CK, NH * HD], bf16, name="wk_t")
            nc.sync.dma_start(wq_t[:], d_wq.ap()[:])
            nc.sync.dma_start(wk_t[:], d_wk.ap()[:])
            bq_t = wp.tile([128, NH], f32, name="bq_t")
            nc.sync.dma_start(bq_t[:], d_bq.ap()[:])
            bk_t = wp.tile([128, NH], f32, name="bk_t")
            nc.sync.dma_start(bk_t[:], d_bk.ap()[:])

            wv_t = wp.tile([128, CK, NH * HD], bf16, name="wv_t")
            wo_t = wp.tile([128, NH, C], bf16, name="wo_t")
            cos_t = wp.tile([128, T], bf16, name="cos_t")
            sin_t = wp.tile([128, T], bf16, name="sin_t")
            masks_t = wp.tile([128, 128], bf16, name="masks_t")
            ones_t = wp.tile([128, 128], bf16, name="ones_t")

            for b in range(B):
                kts = [
                    kvp.tile([128, T], bf16, tag=f"kt{h}", bufs=2, name=f"kt{h}_{b}")
                    for h in range(NH)
                ]
                vt = kvp.tile([128, CK, NH * HD], bf16, tag="v", bufs=2, name=f"v_{b}")

                for tb in range(NTB):
                    tbs = slice(tb * TB, (tb + 1) * TB)
                    # ---- stream x block (4 DMAs of [128, 4, 512]) ----
                    xts = []
                    for j in range(4):
                        xt = xp.tile([128, 4, TB], bf16, tag="xt", bufs=12)
                        # block 0: alternate queues so quarters arrive in
                        # parallel with the wq/wk loads
                        eng = (
                            (nc.sync if j % 2 == 0 else nc.gpsimd)
                            if (b == 0 and tb == 0)
                            else nc.gpsimd
                        )
                        eng.dma_start(
                            xt[:], d_x.ap()[b, :, 4 * j : 4 * j + 4, tbs]
                        )
                        xts.append(xt)
                    if b == 0 and tb == 0:
                        # cos/sin needed at the first RoPE (~10us in); wv at
                        # the first v-proj
                        nc.gpsimd.dma_start(cos_t[:], d_cos.ap()[:])
                        nc.gpsimd.dma_start(sin_t[:], d_sin.ap()[:])
                        nc.sync.dma_start(masks_t[:], d_mask.ap()[:])
                        nc.sync.dma_start(ones_t[:], d_ones.ap()[:])
                        nc.gpsimd.dma_start(wv_t[:], d_wv.ap()[:])
                        # wo last on the queue: only needed at the first
                        # output projection (~40us in)
                        nc.gpsimd.dma_start(wo_t[:], d_wo.ap()[:])

                    def xc(kc):
                        return xts[kc // 4][:, kc % 4, :]

                    # ---- q/k projections + RoPE for both heads ----
                    qTs = []
                    for h in range(NH):
                        hsl = slice(h * HD, (h + 1) * HD)
                        qT = wk_.tile([128, TB], bf16, tag=f"q{h}", bufs=2)
                        for (w_t, b_t, dest) in (
                            (wq_t, bq_t, qT),
                            (wk_t, bk_t, kts[h][:, tbs]),
                        ):
                            prj = ps.tile([128, TB], f32, tag="mm", bufs=2)
                            for kc in range(CK):
                                nc.tensor.matmul(
                                    prj[:],
                                    w_t[:, kc, hsl],
                                    xc(kc),
                                    start=(kc == 0),
                                    stop=(kc == CK - 1),
                                )
                            qb = wk_.tile([128, TB], bf16, tag="qb", bufs=3)
                            nc.scalar.activation(
                                qb[:], prj[:], AF.Identity, bias=b_t[:, h : h + 1]
                            )
                            rotb = wk_.tile([128, TB], bf16, tag="rot", bufs=3)
                            nc.sync.dma_start(rotb[0:64, :], qb[64:128, :])
                            nc.sync.dma_start(rotb[64:128, :], qb[0:64, :])
                            t1 = wk_.tile([128, TB], bf16, tag="t1", bufs=3)
                            nc.vector.tensor_tensor(
                                t1[:], qb[:], cos_t[:, tbs], OP.mult
                            )
                            t2 = wk_.tile([128, TB], bf16, tag="t2", bufs=3)
                            nc.vector.tensor_tensor(
                                t2[:], rotb[:], sin_t[:, tbs], OP.mult
                            )
                            nc.vector.tensor_tensor(dest, t1[:], t2[:], OP.add)
                        qTs.append(qT)

                    # ---- v projection (both heads together, N=256) ----
                    for tt in range(4):
                        vps = ps.tile([128, NH * HD], f32, tag="mm", bufs=2)
                        for kc in range(CK):
                            nc.tensor.matmul(
                                vps[:],
                                xc(kc)[:, tt * 128 : (tt + 1) * 128],
                                wv_t[:, kc, :],
                                start=(kc == 0),
                                stop=(kc == CK - 1),
                            )
                        if tt % 2 == 0:
                            nc.scalar.activation(
                                vt[:, tb * 4 + tt, :], vps[:], AF.Identity
                            )
                        else:
                            nc.vector.tensor_copy(vt[:, tb * 4 + tt, :], vps[:])

                    # ---- attention for this query block ----
                    # heads interleaved per key tile for a denser PE stream
                    nkt = 4 * tb + 4
                    y_pss = [
                        ps.tile([128, TB], f32, tag="my", bufs=2, name=f"yps{h}")
                        for h in range(NH)
                    ]
                    daccs = [
                        wk_.tile([128, TB], bf16, tag="dacc", bufs=4, name=f"dacc{h}")
                        for h in range(NH)
                    ]
                    pt0s = [None] * NH
                    for kt in range(nkt):
                        # causal trim: key tile kt only reaches queries
                        # >= q0 = 128*(kt-4*tb); skip the dead columns
                        o = kt - 4 * tb
                        q0 = 128 * o if o > 0 else 0
                        W = TB - q0
                        for h in range(NH):
                            hsl = slice(h * HD, (h + 1) * HD)
                            s_ps = ps.tile([128, TB], f32, tag="ms", bufs=2)
                            nc.tensor.matmul(
                                s_ps[:, :W],
                                kts[h][:, kt * 128 : (kt + 1) * 128],
                                qTs[h][:, q0:],
                            )
                            pt = wk_.tile([128, TB], bf16, tag="p", bufs=8)
                            nc.scalar.activation(
                                pt[:, :W], s_ps[:, :W], AF.Exp, scale=SCALE
                            )
                            if o >= 0:
                                # triangular chunk is always the first 128
                                # live columns; its mask equals mask0
                                nc.vector.tensor_tensor(
                                    pt[:, :128],
                                    pt[:, :128],
                                    masks_t[:],
                                    OP.mult,
                                )
                            nc.tensor.matmul(
                                y_pss[h][:, q0:],
                                vt[:, kt, hsl],
                                pt[:, :W],
                                start=(kt == 0),
                                stop=(kt == nkt - 1),
                                skip_group_check=True,
                            )
                            if kt == 0:
                                if tb == 0:
                                    # kt=1 is diagonal here; no fusion
                                    nc.vector.tensor_copy(daccs[h][:], pt[:])
                                else:
                                    pt0s[h] = pt
                            elif kt == 1 and tb > 0:
                                # both tiles full: fused init
                                nc.vector.tensor_tensor(
                                    daccs[h][:], pt0s[h][:], pt[:], OP.add
                                )
                            else:
                                nc.vector.tensor_tensor(
                                    daccs[h][:, q0:],
                                    daccs[h][:, q0:],
                                    pt[:, :W],
                                    OP.add,
                                )
                    yTs = []
                    for h in range(NH):
                        den_ps = ps.tile([128, TB], f32, tag="mm", bufs=2)
                        nc.tensor.matmul(den_ps[:], ones_t[:], daccs[h][:])
                        rden = wk_.tile([128, TB], f32, tag="rden", bufs=2)
                        nc.vector.reciprocal_approx_fast(rden[:], den_ps[:])
                        yT = wk_.tile([128, TB], bf16, tag="y", bufs=3)
                        nc.vector.tensor_tensor(yT[:], y_pss[h][:], rden[:], OP.mult)
                        yTs.append(yT)
                    # ---- partial output projection ----
                    # ncc pairs with h outer: each yT stationary feeds two
                    # consecutive matmuls (halves the LDWEIGHTS count here)
                    for tt in range(4):
                        r0 = tb * TB + tt * 128
                        ot = wk_.tile([128, 4, TB], bf16, tag="o", bufs=2)
                        for pair in range(2):
                            o_pss = [
                                ps.tile([128, TB], f32, tag="mo", bufs=2, name=f"ops{k2}")
                                for k2 in range(2)
                            ]
                            for h in range(NH):
                                for k2 in range(2):
                                    ncc = 2 * pair + k2
                                    nc.tensor.matmul(
                                        o_pss[k2][:],
                                        yTs[h][:, tt * 128 : (tt + 1) * 128],
                                        wo_t[:, h, ncc * TB : (ncc + 1) * TB],
                                        start=(h == 0),
                                        stop=(h == NH - 1),
                                        skip_group_check=True,
                                    )
                            for k2 in range(2):
                                ncc = 2 * pair + k2
                                if ncc % 2 == 0:
                                    nc.scalar.activation(
                                        ot[:, ncc, :], o_pss[k2][:], AF.Identity
                                    )
                                else:
                                    nc.vector.tensor_copy(
                                        ot[:, ncc, :], o_pss[k2][:]
                                    )
                        nc.sync.dma_start(
                            d_out.ap()[b, r0 : r0 + 128, :],
                            ot[:],
                        )

    nc.compile()
    return nc


def _get_program():
    if "nc" not in _STATE:
        _STATE["nc"] = _build_program()
    return _STATE["nc"]


def _enable_trace_hooks():
    import types

    import antenv

    if not hasattr(antenv, "axon_hooks"):
        hooks_mod = types.ModuleType("antenv.axon_hooks")
        _hook = [None]
        hooks_mod.set_axon_ntff_profile_hook = lambda h: _hook.__setitem__(0, h)
        hooks_mod.get_axon_ntff_profile_hook = lambda: _hook[0]
        sys.modules["antenv.axon_hooks"] = hooks_mod
        antenv.axon_hooks = hooks_mod
        from trn_agent_boot.trn_boot import _ntff_profile_via_ctypes

        hooks_mod.set_axon_ntff_profile_hook(
            _ntff_profile_via_ctypes("/opt/axon/libaxon_pjrt.so")
        )
    bass_utils.upload_artifacts = lambda tmpdir: f"local://{tmpdir}"


def kernel(x, Wqkv, bqkv, Wo, bo):
    global LAST_RESULT
    x = np.asarray(x, dtype=np.float32)
    Wqkv = np.asarray(Wqkv, dtype=np.float32)
    bqkv = np.asarray(bqkv, dtype=np.float32)
    Wo = np.asarray(Wo, dtype=np.float32)
    bo = np.asarray(bo, dtype=np.float32)

    nc = _get_program()

    cosT, sinT = _rope_tables()
    sinT = sinT.copy()
    sinT[: HD // 2, :] *= -1.0  # rotation sign folded into the sin table
    cosT = cosT.astype(BF)
    sinT = sinT.astype(BF)
    onesm = np.ones((128, 128), dtype=BF)
    # mask0[j, i] = 1 if key j <= query i within the diagonal 128-tile
    i_idx = np.arange(128)[None, :]
    j_idx = np.arange(128)[:, None]
    mask0 = (j_idx <= i_idx).astype(BF)
    # x -> [B, 128, CK, T]: partition p = c % 128 within chunk kc = c // 128
    xT = x.transpose(0, 2, 1)  # [B, C, T]
    xR = np.ascontiguousarray(
        xT.reshape(B, CK, 128, T).transpose(0, 2, 1, 3)
    ).astype(BF)

    def wtile(w):  # [rows=256, C] -> [128, CK, 256] (lhsT chunks)
        return np.ascontiguousarray(
            w.T.reshape(CK, 128, NH * HD).transpose(1, 0, 2)
        ).astype(BF)

    in_maps = []
    for c in range(NC_):
        rs = slice(c * NH * HD, (c + 1) * NH * HD)
        wo_c = Wo[:, rs].T  # [256, C]
        in_maps.append(
            {
                "xR": xR,
                "wq": wtile(Wqkv[0 * C :][rs.start : rs.stop, :]),
                "wk": wtile(Wqkv[1 * C :][rs.start : rs.stop, :]),
                "wv": wtile(Wqkv[2 * C :][rs.start : rs.stop, :]),
                "wo": np.ascontiguousarray(
                    wo_c.reshape(NH, 128, C).transpose(1, 0, 2)
                ).astype(BF),
                "bq": np.ascontiguousarray(bqkv[0 * C :][rs].reshape(NH, HD).T),
                "bk": np.ascontiguousarray(bqkv[1 * C :][rs].reshape(NH, HD).T),
                "cosT": cosT,
                "sinT": sinT,
                "mask0": mask0,
                "onesm": onesm,
            }
        )

    if TRACE:
        _enable_trace_hooks()
    res = bass_utils.run_bass_kernel_spmd(
        nc, in_maps, core_ids=list(range(NC_)), trace=TRACE
    )
    LAST_RESULT = res

    out = np.zeros((B, T, C), dtype=np.float64)
    for c in range(NC_):
        out += res.results[c]["out"].astype(np.float32)
    bv = bqkv[2 * C : 3 * C]
    out += (bo + Wo @ bv)[None, None, :]
    return out.astype(np.float32)


# revision 14
# speedup vs baseline: 1.6675x; 1.0618x over previous
"""Causal self-attention (B=2, T=2048, C=2048, H=16) on 8 TRN2 NeuronCores.

Sharding: tensor-parallel over heads (2 heads per core, both batches on every
core). Each core computes q/k/v projections for its 2 heads, RoPE, causal
softmax(qk^T)v, and a partial output projection against its slice of Wo's
columns. The host sums the 8 partial projections and adds the (linear) bias
terms.

v2: all matmul operands are bf16 (PSUM accumulation stays fp32). fp32
LDWEIGHTS gets no Fast Weight Load and dominated the v1 PE timeline; bf16
halves both the weight-load path and all DMA traffic. Inputs are pre-cast
and pre-tiled on host so every weight/x transfer is one large DMA.

Layout on device:
  - x is host-relaid to xR[b] = [128, CK, T] (partition = c%128within chunk);
    contraction over C has C on partitions for both operands.
  - q, k are produced transposed: qT/kT [head_dim, T]. Scores are
    S_T = kT_tile.T @ qT [keys, queries]; softmax normalization runs along
    the partition (key) dim.
  - exp via ACT (scale = 1/sqrt(HD) fused), bf16 out; causal masking via a
    0/1 bf16 mask multiply on the diagonal-boundary tiles.
  - denominator: P_T tiles are accumulated on DVE (bf16, 2x mode), then one
    matmul with an all-ones [128,128] lhsT broadcasts fp32 column sums.
  - y is produced transposed: yT = v_tile.T @ P_T; normalization multiplies
    by the broadcast reciprocal.
  - output projection: out_tile = yT_slice.T @ woT_slice; a [128, 2048]
    bf16 staging tile per row block keeps the writeback to 1 DMA.
  - RoPE rotation (half-swap, sign folded into the sin table) is 2 SBUF
    partition-swap DMAs + 3 bf16 DVE multiplies/adds.
"""

import sys

sys.path.insert(0, "/opt/trn_rl_repo")

import ml_dtypes
import numpy as np

import concourse.bacc as bacc
import concourse.mybir as mybir
import concourse.tile as tile
from concourse import bass_utils

B, T, C, H = 2, 2048, 2048, 16
HD = C // H  # 128
BASE = 10000.0
NC_ = 8  # cores
NH = H // NC_  # heads per core = 2
TB = 512  # T block
NTB = T // TB  # 4
CK = C // 128  # 16 contraction chunks
SCALE = 1.0 / float(np.sqrt(np.float32(HD)))

f32 = mybir.dt.float32
bf16 = mybir.dt.bfloat16
AF = mybir.ActivationFunctionType
OP = mybir.AluOpType
BF = ml_dtypes.bfloat16

TRACE = False
LAST_RESULT = None
LDW_OPT = False

_orig_run_command = bass_utils.run_command


def _patched_run_command(cmd, **kw):
    if LDW_OPT and isinstance(cmd, list):
        cmd = [
            ("--enable-ldw-opt=true" if c == "--enable-ldw-opt=false" else c)
            for c in cmd
        ]
    return _orig_run_command(cmd, **kw)


bass_utils.run_command = _patched_run_command

_STATE = {}


def _rope_tables():
    """cos/sin tables [HD, T] mirroring reference._rope_tables (f32 chain)."""
    try:
        import jax
        import jax.numpy as jnp

        cpu = jax.devices("cpu")[0]
        with jax.default_device(cpu):
            p = jnp.arange(HD // 2, dtype=jnp.float32)
            theta = jnp.power(BASE, -(2.0**p) / HD)
            pos = jnp.arange(1, T + 1, dtype=jnp.float32)[:, None]
            c = pos * theta
            ang = jnp.concatenate([c, c], axis=-1)  # [T, HD]
            cos = np.asarray(jnp.cos(ang)).T  # [HD, T]
            sin = np.asarray(jnp.sin(ang)).T
        return np.ascontiguousarray(cos), np.ascontiguousarray(sin)
    except Exception:
        p = np.arange(HD // 2, dtype=np.float32)
        theta = np.power(np.float32(BASE), (-(2.0**p) / HD).astype(np.float32))
        pos = np.arange(1, T + 1, dtype=np.float32)[:, None]
        c = (pos * theta).astype(np.float32)
        ang = np.concatenate([c, c], axis=-1)
        return (
            np.ascontiguousarray(np.cos(ang).T.astype(np.float32)),
            np.ascontiguousarray(np.sin(ang).T.astype(np.float32)),
        )


def _build_program():
    nc = bacc.Bacc("TRN2", target_bir_lowering=False, debug=False, num_devices=NC_)

    d_x = nc.dram_tensor("xR", (B, 128, CK, T), bf16, kind="ExternalInput")
    d_wq = nc.dram_tensor("wq", (128, CK, NH * HD), bf16, kind="ExternalInput")
    d_wk = nc.dram_tensor("wk", (128, CK, NH * HD), bf16, kind="ExternalInput")
    d_wv = nc.dram_tensor("wv", (128, CK, NH * HD), bf16, kind="ExternalInput")
    d_wo = nc.dram_tensor("wo", (128, NH, C), bf16, kind="ExternalInput")
    d_bq = nc.dram_tensor("bq", (HD, NH), f32, kind="ExternalInput")
    d_bk = nc.dram_tensor("bk", (HD, NH), f32, kind="ExternalInput")
    d_cos = nc.dram_tensor("cosT", (HD, T), bf16, kind="ExternalInput")
    d_sin = nc.dram_tensor("sinT", (HD, T), bf16, kind="ExternalInput")
    d_mask = nc.dram_tensor("mask0", (128, 128), bf16, kind="ExternalInput")
    d_ones = nc.dram_tensor("onesm", (128, 128), bf16, kind="ExternalInput")
    d_out = nc.dram_tensor("out", (B, T, C), bf16, kind="ExternalOutput")

    with tile.TileContext(nc) as tc:
        with (
            tc.tile_pool(name="w", bufs=1) as wp,
            tc.tile_pool(name="xp", bufs=1) as xp,
            tc.tile_pool(name="kv", bufs=1) as kvp,
            tc.tile_pool(name="work", bufs=1) as wk_,
            tc.tile_pool(name="ps", bufs=1, space="PSUM") as ps,
        ):
            # --- resident weights/constants ---
            wq_t = wp.tile([128, CK, NH * HD], bf16, name="wq_t")
            wk_t = wp.tile([128, CK, NH * HD], bf16, name="wk_t")
            nc.sync.dma_start(wq_t[:], d_wq.ap()[:])
            nc.sync.dma_start(wk_t[:], d_wk.ap()[:])
            bq_t = wp.tile([128, NH], f32, name="bq_t")
            nc.sync.dma_start(bq_t[:], d_bq.ap()[:])
            bk_t = wp.tile([128, NH], f32, name="bk_t")
            nc.sync.dma_start(bk_t[:], d_bk.ap()[:])

            wv_t = wp.tile([128, CK, NH * HD], bf16, name="wv_t")
            wo_t = wp.tile([128, NH, C], bf16, name="wo_t")
            cos_t = wp.tile([128, T], bf16, name="cos_t")
            sin_t = wp.tile([128, T], bf16, name="sin_t")
            masks_t = wp.tile([128, 128], bf16, name="masks_t")
            ones_t = wp.tile([128, 128], bf16, name="ones_t")

            for b in range(B):
                kts = [
                    kvp.tile([128, T], bf16, tag=f"kt{h}", bufs=2, name=f"kt{h}_{b}")
                    for h in range(NH)
                ]
                vt = kvp.tile([128, CK, NH * HD], bf16, tag="v", bufs=2, name=f"v_{b}")

                for tb in range(NTB):
                    tbs = slice(tb * TB, (tb + 1) * TB)
                    # ---- stream x block (4 DMAs of [128, 4, 512]) ----
                    # All input loads share the sync queue (strict FIFO =
                    # bandwidth priority). Each block's RoPE-swap DMAs sit
                    # between this block's loads and the next block's
                    # prefetch in the FIFO, so prefetch is naturally paced
                    # and can't starve the current block's critical bytes.
                    xts = []
                    for j in range(4):
                        xt = xp.tile([128, 4, TB], bf16, tag="xt", bufs=8)
                        nc.sync.dma_start(
                            xt[:], d_x.ap()[b, :, 4 * j : 4 * j + 4, tbs]
                        )
                        xts.append(xt)
                    if b == 0 and tb == 0:
                        # in need-order: cos/sin at first RoPE, wv at first
                        # v-proj, masks/ones at first attention
                        nc.sync.dma_start(cos_t[:], d_cos.ap()[:])
                        nc.sync.dma_start(sin_t[:], d_sin.ap()[:])
                        nc.sync.dma_start(wv_t[:], d_wv.ap()[:])
                        nc.sync.dma_start(masks_t[:], d_mask.ap()[:])
                        nc.sync.dma_start(ones_t[:], d_ones.ap()[:])
                        # wo last: only needed at the first output
                        # projection (end of block 0)
                        nc.sync.dma_start(wo_t[:], d_wo.ap()[:])

                    def xc(kc):
                        return xts[kc // 4][:, kc % 4, :]

                    # ---- q/k projections + RoPE for both heads ----
                    qTs = []
                    for h in range(NH):
                        hsl = slice(h * HD, (h + 1) * HD)
                        qT = wk_.tile([128, TB], bf16, tag=f"q{h}", bufs=2)
                        for (w_t, b_t, dest) in (
                            (wq_t, bq_t, qT),
                            (wk_t, bk_t, kts[h][:, tbs]),
                        ):
                            prj = ps.tile([128, TB], f32, tag="mm", bufs=2)
                            for kc in range(CK):
                                nc.tensor.matmul(
                                    prj[:],
                                    w_t[:, kc, hsl],
                                    xc(kc),
                                    start=(kc == 0),
                                    stop=(kc == CK - 1),
                                )
                            qb = wk_.tile([128, TB], bf16, tag="qb", bufs=3)
                            nc.scalar.activation(
                                qb[:], prj[:], AF.Identity, bias=b_t[:, h : h + 1]
                            )
                            rotb = wk_.tile([128, TB], bf16, tag="rot", bufs=3)
                            nc.sync.dma_start(rotb[0:64, :], qb[64:128, :])
                            nc.sync.dma_start(rotb[64:128, :], qb[0:64, :])
                            t1 = wk_.tile([128, TB], bf16, tag="t1", bufs=3)
                            nc.vector.tensor_tensor(
                                t1[:], qb[:], cos_t[:, tbs], OP.mult
                            )
                            t2 = wk_.tile([128, TB], bf16, tag="t2", bufs=3)
                            nc.vector.tensor_tensor(
                                t2[:], rotb[:], sin_t[:, tbs], OP.mult
                            )
                            nc.vector.tensor_tensor(dest, t1[:], t2[:], OP.add)
                        qTs.append(qT)

                    # ---- v projection (both heads together, N=256) ----
                    for tt in range(4):
                        vps = ps.tile([128, NH * HD], f32, tag="mm", bufs=2)
                        for kc in range(CK):
                            nc.tensor.matmul(
                                vps[:],
                                xc(kc)[:, tt * 128 : (tt + 1) * 128],
                                wv_t[:, kc, :],
                                start=(kc == 0),
                                stop=(kc == CK - 1),
                            )
                        if tt % 2 == 0:
                            nc.scalar.activation(
                                vt[:, tb * 4 + tt, :], vps[:], AF.Identity
                            )
                        else:
                            nc.vector.tensor_copy(vt[:, tb * 4 + tt, :], vps[:])

                    # ---- attention for this query block ----
                    # heads interleaved per key tile for a denser PE stream
                    nkt = 4 * tb + 4
                    y_pss = [
                        ps.tile([128, TB], f32, tag="my", bufs=2, name=f"yps{h}")
                        for h in range(NH)
                    ]
                    daccs = [
                        wk_.tile([128, TB], bf16, tag="dacc", bufs=4, name=f"dacc{h}")
                        for h in range(NH)
                    ]
                    pt0s = [None] * NH
                    for kt in range(nkt):
                        # causal trim: key tile kt only reaches queries
                        # >= q0 = 128*(kt-4*tb); skip the dead columns
                        o = kt - 4 * tb
                        q0 = 128 * o if o > 0 else 0
                        W = TB - q0
                        for h in range(NH):
                            hsl = slice(h * HD, (h + 1) * HD)
                            s_ps = ps.tile([128, TB], f32, tag="ms", bufs=2)
                            nc.tensor.matmul(
                                s_ps[:, :W],
                                kts[h][:, kt * 128 : (kt + 1) * 128],
                                qTs[h][:, q0:],
                            )
                            pt = wk_.tile([128, TB], bf16, tag="p", bufs=8)
                            nc.scalar.activation(
                                pt[:, :W], s_ps[:, :W], AF.Exp, scale=SCALE
                            )
                            if o >= 0:
                                # triangular chunk is always the first 128
                                # live columns; its mask equals mask0
                                nc.vector.tensor_tensor(
                                    pt[:, :128],
                                    pt[:, :128],
                                    masks_t[:],
                                    OP.mult,
                                )
                            nc.tensor.matmul(
                                y_pss[h][:, q0:],
                                vt[:, kt, hsl],
                                pt[:, :W],
                                start=(kt == 0),
                                stop=(kt == nkt - 1),
                                skip_group_check=True,
                            )
                            if kt == 0:
                                if tb == 0:
                                    # kt=1 is diagonal here; no fusion
                                    nc.vector.tensor_copy(daccs[h][:], pt[:])
                                else:
                                    pt0s[h] = pt
                            elif kt == 1 and tb > 0:
                                # both tiles full: fused init
                                nc.vector.tensor_tensor(
                                    daccs[h][:], pt0s[h][:], pt[:], OP.add
                                )
                            else:
                                nc.vector.tensor_tensor(
                                    daccs[h][:, q0:],
                                    daccs[h][:, q0:],
                                    pt[:, :W],
                                    OP.add,
                                )
                    yTs = []
                    for h in range(NH):
                        den_ps = ps.tile([128, TB], f32, tag="mm", bufs=2)
                        nc.tensor.matmul(den_ps[:], ones_t[:], daccs[h][:])
                        rden = wk_.tile([128, TB], f32, tag="rden", bufs=2)
                        nc.vector.reciprocal_approx_fast(rden[:], den_ps[:])
                        yT = wk_.tile([128, TB], bf16, tag="y", bufs=3)
                        nc.vector.tensor_tensor(yT[:], y_pss[h][:], rden[:], OP.mult)
                        yTs.append(yT)
                    # ---- partial output projection ----
                    # ncc pairs with h outer: each yT stationary feeds two
                    # consecutive matmuls (halves the LDWEIGHTS count here)
                    for tt in range(4):
                        r0 = tb * TB + tt * 128
                        ot = wk_.tile([128, 4, TB], bf16, tag="o", bufs=2)
                        for pair in range(2):
                            o_pss = [
                                ps.tile([128, TB], f32, tag="mo", bufs=2, name=f"ops{k2}")
                                for k2 in range(2)
                            ]
                            for h in range(NH):
                                for k2 in range(2):
                                    ncc = 2 * pair + k2
                                    nc.tensor.matmul(
                                        o_pss[k2][:],
                                        yTs[h][:, tt * 128 : (tt + 1) * 128],
                                        wo_t[:, h, ncc * TB : (ncc + 1) * TB],
                                        start=(h == 0),
                                        stop=(h == NH - 1),
                                        skip_group_check=True,
                                    )
                            for k2 in range(2):
                                ncc = 2 * pair + k2
                                if ncc % 2 == 0:
                                    nc.scalar.activation(
                                        ot[:, ncc, :], o_pss[k2][:], AF.Identity
                                    )
                                else:
                                    nc.vector.tensor_copy(
                                        ot[:, ncc, :], o_pss[k2][:]
                                    )
                        nc.scalar.dma_start(
                            d_out.ap()[b, r0 : r0 + 128, :],
                            ot[:],
                        )

    nc.compile()
    return nc


def _get_program():
    if "nc" not in _STATE:
        _STATE["nc"] = _build_program()
    return _STATE["nc"]


def _enable_trace_hooks():
    import types

    import antenv

    if not hasattr(antenv, "axon_hooks"):
        hooks_mod = types.ModuleType("antenv.axon_hooks")
        _hook = [None]
        hooks_mod.set_axon_ntff_profile_hook = lambda h: _hook.__setitem__(0, h)
        hooks_mod.get_axon_ntff_profile_hook = lambda: _hook[0]
        sys.modules["antenv.axon_hooks"] = hooks_mod
        antenv.axon_hooks = hooks_mod
        from trn_agent_boot.trn_boot import _ntff_profile_via_ctypes

        hooks_mod.set_axon_ntff_profile_hook(
            _ntff_profile_via_ctypes("/opt/axon/libaxon_pjrt.so")
        )
    bass_utils.upload_artifacts = lambda tmpdir: f"local://{tmpdir}"


def kernel(x, Wqkv, bqkv, Wo, bo):
    global LAST_RESULT
    x = np.asarray(x, dtype=np.float32)
    Wqkv = np.asarray(Wqkv, dtype=np.float32)
    bqkv = np.asarray(bqkv, dtype=np.float32)
    Wo = np.asarray(Wo, dtype=np.float32)
    bo = np.asarray(bo, dtype=np.float32)

    nc = _get_program()

    cosT, sinT = _rope_tables()
    sinT = sinT.copy()
    sinT[: HD // 2, :] *= -1.0  # rotation sign folded into the sin table
    cosT = cosT.astype(BF)
    sinT = sinT.astype(BF)
    onesm = np.ones((128, 128), dtype=BF)
    # mask0[j, i] = 1 if key j <= query i within the diagonal 128-tile
    i_idx = np.arange(128)[None, :]
    j_idx = np.arange(128)[:, None]
    mask0 = (j_idx <= i_idx).astype(BF)
    # x -> [B, 128, CK, T]: partition p = c % 128 within chunk kc = c // 128
    xT = x.transpose(0, 2, 1)  # [B, C, T]
    xR = np.ascontiguousarray(
        xT.reshape(B, CK, 128, T).transpose(0, 2, 1, 3)
    ).astype(BF)

    def wtile(w):  # [rows=256, C] -> [128, CK, 256] (lhsT chunks)
        return np.ascontiguousarray(
            w.T.reshape(CK, 128, NH * HD).transpose(1, 0, 2)
        ).astype(BF)

    in_maps = []
    for c in range(NC_):
        rs = slice(c * NH * HD, (c + 1) * NH * HD)
        wo_c = Wo[:, rs].T  # [256, C]
        in_maps.append(
            {
                "xR": xR,
                "wq": wtile(Wqkv[0 * C :][rs.start : rs.stop, :]),
                "wk": wtile(Wqkv[1 * C :][rs.start : rs.stop, :]),
                "wv": wtile(Wqkv[2 * C :][rs.start : rs.stop, :]),
                "wo": np.ascontiguousarray(
                    wo_c.reshape(NH, 128, C).transpose(1, 0, 2)
                ).astype(BF),
                "bq": np.ascontiguousarray(bqkv[0 * C :][rs].reshape(NH, HD).T),
                "bk": np.ascontiguousarray(bqkv[1 * C :][rs].reshape(NH, HD).T),
                "cosT": cosT,
                "sinT": sinT,
                "mask0": mask0,
                "onesm": onesm,
            }
        )

    if TRACE:
        _enable_trace_hooks()
    res = bass_utils.run_bass_kernel_spmd(
        nc, in_maps, core_ids=list(range(NC_)), trace=TRACE
    )
    LAST_RESULT = res

    out = np.zeros((B, T, C), dtype=np.float64)
    for c in range(NC_):
        out += res.results[c]["out"].astype(np.float32)
    bv = bqkv[2 * C : 3 * C]
    out += (bo + Wo @ bv)[None, None, :]
    return out.astype(np.float32)
